# revision 14
# baseline (speedup 1.0000x reference)
"""Trainium2 Bass kernel for nn_EncoderLayer_64321430225657 (ProbSparse encoder layer).

Self-contained: hardcodes shapes/sharding. Data-parallel over batch B=8 across
8 NeuronCores (one batch element per core). All sparse gathers/scatters are
reformulated as dense matmuls using host-precomputed index-derived matrices and
device-built one-hot selection matrices. The M-measure path (which decides the
exact top-35 query indices, part of the output) runs in split-fp32r 3-pass
matmuls for fp32-exact results; everything downstream of the top-k runs 1-pass
fp32r (~1.2e-4 relative error).
"""
import sys
import os

for _p in ("/opt/trn_rl_repo",):
    if _p not in sys.path and os.path.isdir(_p):
        sys.path.insert(0, _p)

import numpy as np
import ml_dtypes

import concourse.bass as bass
import concourse.tile as tile
import concourse.mybir as mybir
from concourse import bass_utils

F32 = mybir.dt.float32
F32R = mybir.dt.float32r
BF16 = mybir.dt.bfloat16
I32 = mybir.dt.int32
U32 = mybir.dt.uint32
AX = mybir.AxisListType
OP = mybir.AluOpType
ACTF = mybir.ActivationFunctionType

B, L, D = 8, 1024, 512
H, DK, DV = 8, 64, 64
U, UP = 35, 36
DFF = 2048
NL, ND, NJ, NF = 8, 4, 8, 16  # 128-chunks of L, D, L(keys), DFF
NEG = -3.0e38
MASKNEG = -1.0e30
GROUPS = ((0, 3), (3, 6), (6, 8))  # head groups for packed softmax/transposes


def split_multi_waits(nc):
    """Walrus codegen in this container supports only one sync-wait per
    instruction; hoist extras onto preceding NOPs on the same engine."""
    for f in nc.m.functions:
        for bb in f.blocks:
            insts = list(bb.instructions)
            new_insts = []
            changed = False
            for ins in insts:
                si = ins.sync_info
                if si is not None and len(si.on_wait) > 1:
                    waits = list(si.on_wait)
                    for k, w in enumerate(waits[:-1]):
                        new_insts.append(mybir.InstNoOp(
                            name=f"{ins.name}-wsplit{k}",
                            sync_info=mybir.SyncInfo(on_wait=[w], on_update=[]),
                            bass_nofuse=True,
                            engine=ins.engine,
                        ))
                    si.on_wait = [waits[-1]]
                    ins.sync_info = si
                    changed = True
                new_insts.append(ins)
            if changed:
                bb.instructions = new_insts
    return nc


def _emit(nc, tc, dbg=False):
    # ---- DRAM I/O ----
    xT = nc.dram_tensor("xT", [D, L], F32, kind="ExternalInput")
    Wq = nc.dram_tensor("Wq", [D, H * DK], F32, kind="ExternalInput")
    Wk = nc.dram_tensor("Wk", [D, H * DK], F32, kind="ExternalInput")
    Wv = nc.dram_tensor("Wv", [D, H * DV], F32, kind="ExternalInput")
    Wo = nc.dram_tensor("Wo", [H * DV, D], F32, kind="ExternalInput")
    c1T = nc.dram_tensor("c1T", [D, DFF], F32, kind="ExternalInput")
    c2T = nc.dram_tensor("c2T", [DFF, D], F32, kind="ExternalInput")
    CntT = nc.dram_tensor("CntT", [L, L], F32, kind="ExternalInput")
    maskadd = nc.dram_tensor("maskadd", [L, L], F32, kind="ExternalInput")
    iota_row = nc.dram_tensor("iota_row", [UP, L], F32, kind="ExternalInput")
    iotac = nc.dram_tensor("iotac", [128, H * UP], F32, kind="ExternalInput")
    ident = nc.dram_tensor("ident", [128, 128], F32, kind="ExternalInput")
    onesr = nc.dram_tensor("onesr", [1, D], F32, kind="ExternalInput")
    onesc = nc.dram_tensor("onesc", [128, 1], F32, kind="ExternalInput")

    OUT = nc.dram_tensor("OUT", [L, D], F32, kind="ExternalOutput")
    MTOP = nc.dram_tensor("MTOP", [H, U], I32, kind="ExternalOutput")
    mscr = nc.dram_tensor("mtop_scratch", [1, H * UP], F32)  # internal bounce
    if dbg:
        DBG_M = nc.dram_tensor("DBG_M", [L, H], F32, kind="ExternalOutput")
        DBG_QT = nc.dram_tensor("DBG_QT", [D, L], F32, kind="ExternalOutput")
        DBG_AO = nc.dram_tensor("DBG_AO", [D, L], F32, kind="ExternalOutput")

    dma = nc.sync.dma_start
    dmac = nc.gpsimd.dma_start  # dtype-casting DMAs (fp32 -> fp32r)

    def pool(name, bufs=1):
        cm = tc.tile_pool(name=name, bufs=bufs)
        return cm, cm.__enter__()

    # =================== constants (live forever) ===================
    cst_cm, cst = pool("cst")
    ident32 = cst.tile([128, 128], F32, name="ident32")
    dma(ident32[:], ident.ap())
    identr = cst.tile([128, 128], F32R, name="identr")
    dmac(identr[:], ident.ap())
    ones_r32 = cst.tile([1, D], F32, name="ones_r32")
    dma(ones_r32[:], onesr.ap())
    ones_rr = cst.tile([1, D], F32R, name="ones_rr")
    dmac(ones_rr[:], onesr.ap())
    ones_cr = cst.tile([128, 1], F32R, name="ones_cr")
    dmac(ones_cr[:], onesc.ap())
    iota_row_sb = cst.tile([UP, L], F32, name="iota_row_sb")
    dma(iota_row_sb[:], iota_row.ap())
    iotac_sb = cst.tile([128, H * UP], F32, name="iotac_sb")
    dma(iotac_sb[:], iotac.ap())

    # ---- mid-life tensor pools (creation order = reverse release order) ----
    vr_cm, vr_p = pool("vrp")            # released after phase 5b
    v_r = [vr_p.tile([128, D], F32R, name=f"vr{i}") for i in range(NL)]
    vmean_r = vr_p.tile([1, D], F32R, name="vmean_r")
    negvm_r = vr_p.tile([1, D], F32R, name="negvm_r")
    kThi_cm, kThi_p = pool("kThi")       # released after phase 5
    kT_hi = [kThi_p.tile([128, L], F32R, name=f"kThi{i}") for i in range(ND)]
    q32_cm, q32_p = pool("q32p")         # released after phase 5
    q32 = [q32_p.tile([128, D], F32, name=f"q32_{i}") for i in range(NL)]
    mtM_cm, mtM_p = pool("mtM")          # released in phase 4
    mt_sb = [mtM_p.tile([128, H], F32, name=f"mt{i}") for i in range(NL)]
    M_sb = [mtM_p.tile([128, H], F32, name=f"M{i}") for i in range(NL)]
    kr_cm, kr_p = pool("krp")            # released after phase 3
    k_r = [kr_p.tile([128, D], F32R, name=f"kr{i}") for i in range(NL)]
    qkx_cm, qkx = pool("qkx")            # released after phase 2
    qT_hi = [qkx.tile([128, L], F32R, name=f"qThi{i}") for i in range(ND)]
    qT_lo = [qkx.tile([128, L], F32R, name=f"qTlo{i}") for i in range(ND)]
    kT_lo = [qkx.tile([128, L], F32R, name=f"kTlo{i}") for i in range(ND)]

    # =================== phase 1: splits + projections ===================
    with tc.tile_pool(name="p1", bufs=1) as p1, \
         tc.tile_pool(name="tmp1", bufs=1) as tmp1, \
         tc.tile_pool(name="ps1", bufs=4, space="PSUM") as ps1, \
         tc.tile_pool(name="psA", bufs=1, space="PSUM") as psA:
        x_hi = [p1.tile([128, L], F32R, name=f"xh{i}") for i in range(ND)]
        x_lo = [p1.tile([128, L], F32R, name=f"xl{i}") for i in range(ND)]
        for i in range(ND):
            t32 = tmp1.tile([128, L], F32, tag="t32x", name=f"xT32_{i}")
            dma(t32[:], xT.ap()[i * 128:(i + 1) * 128, :])
            nc.scalar.copy(x_hi[i][:], t32[:])
            nc.vector.tensor_tensor(
                out=x_lo[i][:], in0=t32[:], in1=x_hi[i][:], op=OP.subtract)

        for nm, Wd, t_hi, t_lo, sink, cvt in (
                ("q", Wq, qT_hi, qT_lo, q32, "dve32"),
                ("k", Wk, kT_hi, kT_lo, k_r, "actr")):
            with tc.tile_pool(name=f"w{nm}", bufs=1) as wp:
                w_hi, w_lo = [], []
                for i in range(ND):
                    w32 = tmp1.tile([128, D], F32, tag="t32w",
                                    name=f"w32_{nm}{i}")
                    dma(w32[:], Wd.ap()[i * 128:(i + 1) * 128, :])
                    hi = wp.tile([128, D], F32R, name=f"whi_{nm}{i}")
                    lo = wp.tile([128, D], F32R, name=f"wlo_{nm}{i}")
                    nc.scalar.copy(hi[:], w32[:])
                    nc.vector.tensor_tensor(out=lo[:], in0=w32[:], in1=hi[:],
                                            op=OP.subtract)
                    w_hi.append(hi)
                    w_lo.append(lo)
                # transposed layout [hd, l]: 3-pass exact -> hi/lo splits
                for dk in range(ND):
                    for l2 in range(2):
                        ps = ps1.tile([128, 512], F32, tag="proj",
                                      name="proj_ps")
                        nmm = 0
                        for dd in range(ND):
                            for wset, xset in ((w_hi, x_hi), (w_hi, x_lo),
                                               (w_lo, x_hi)):
                                nc.tensor.matmul(
                                    ps[:],
                                    wset[dd][:, dk * 128:(dk + 1) * 128],
                                    xset[dd][:, l2 * 512:(l2 + 1) * 512],
                                    start=(nmm == 0), stop=(nmm == 11))
                                nmm += 1
                        hi_sl = t_hi[dk][:, l2 * 512:(l2 + 1) * 512]
                        lo_sl = t_lo[dk][:, l2 * 512:(l2 + 1) * 512]
                        nc.scalar.copy(hi_sl, ps[:])
                        nc.vector.tensor_tensor(out=lo_sl, in0=ps[:],
                                                in1=hi_sl, op=OP.subtract)
                # normal layout [l, hd]: 3-pass exact
                for lt in range(NL):
                    ps = ps1.tile([128, 512], F32, tag="proj", name="qkn_ps")
                    nmm = 0
                    for dd in range(ND):
                        for wset, xset in ((w_hi, x_hi), (w_hi, x_lo),
                                           (w_lo, x_hi)):
                            nc.tensor.matmul(
                                ps[:],
                                xset[dd][:, lt * 128:(lt + 1) * 128],
                                wset[dd][:],
                                start=(nmm == 0), stop=(nmm == 11))
                            nmm += 1
                    if cvt == "dve32":
                        nc.vector.tensor_copy(sink[lt][:], ps[:])
                    else:
                        nc.scalar.copy(sink[lt][:], ps[:])
        if dbg:
            for dk in range(ND):
                dmac(DBG_QT.ap()[dk * 128:(dk + 1) * 128, :], qT_hi[dk][:])

        # v: 1-pass fp32r, normal layout [l, hd]
        with tc.tile_pool(name="wv", bufs=1) as wvp:
            Wv_r = []
            for i in range(ND):
                wv = wvp.tile([128, D], F32R, name=f"wv{i}")
                dmac(wv[:], Wv.ap()[i * 128:(i + 1) * 128, :])
                Wv_r.append(wv)
            for lt in range(NL):
                ps = ps1.tile([128, 512], F32, tag="proj", name="v_ps")
                for dd in range(ND):
                    nc.tensor.matmul(
                        ps[:],
                        x_hi[dd][:, lt * 128:(lt + 1) * 128],
                        Wv_r[dd][:],
                        start=(dd == 0), stop=(dd == ND - 1))
                nc.scalar.copy(v_r[lt][:], ps[:])

        # vmean row [1, 512] and its negation (fp32r rows for K=1 matmuls)
        psv = psA.tile([1, 512], F32, tag="vm", name="vm_ps")
        for lt in range(NL):
            nc.tensor.matmul(psv[:], ones_cr[:], v_r[lt][:],
                             start=(lt == 0), stop=(lt == NL - 1))
        nc.scalar.mul(vmean_r[:], psv[:], 1.0 / L)
        nc.scalar.mul(negvm_r[:], psv[:], -1.0 / L)

    # =================== phase 2: QK + masked max ===================
    # The sampling mask (0 / -1e30) is injected into PSUM via an identity
    # matmul as the first accumulation step; a single reduce_max per
    # (head, l-tile) then yields the masked max.
    with tc.tile_pool(name="mask", bufs=2) as maskp, \
         tc.tile_pool(name="psQK", bufs=3, space="PSUM") as psQK:
        for lt in range(NL):
            mtile = maskp.tile([128, L], F32R, tag="mask", name=f"mask{lt}")
            dmac(mtile[:], maskadd.ap()[lt * 128:(lt + 1) * 128, :])
            lsl = slice(lt * 128, (lt + 1) * 128)
            for hp in range(4):  # heads (2hp, 2hp+1) live in qT tile hp
                for hh in range(2):
                    h = 2 * hp + hh
                    rsl = slice(hh * 64, hh * 64 + 64)
                    tp = (hh * 64, 0)
                    psh = psQK.tile([128, L], F32, tag="qk", name=f"qk{h}")
                    for j2 in range(2):
                        jsl = slice(j2 * 512, (j2 + 1) * 512)
                        nc.tensor.matmul(
                            psh[:, jsl], identr[:], mtile[:, jsl],
                            start=True, stop=False)
                        for pa, pb in ((qT_hi, kT_hi), (qT_hi, kT_lo),
                                       (qT_lo, kT_hi)):
                            nc.tensor.matmul(
                                psh[:, jsl],
                                pa[hp][rsl, lsl],
                                pb[hp][rsl, jsl],
                                start=False, stop=(pb is kT_hi and pa is qT_lo),
                                tile_position=tp)
                    nc.vector.reduce_max(
                        mt_sb[lt][:, h:h + 1], psh[:], axis=AX.X)
    qkx_cm.__exit__(None, None, None)

    # =================== phase 3: kbar + mean -> M ===================
    with tc.tile_pool(name="cnt", bufs=2) as cntp, \
         tc.tile_pool(name="kbp", bufs=1) as kbp, \
         tc.tile_pool(name="escr", bufs=2) as escrp, \
         tc.tile_pool(name="psKB", bufs=1, space="PSUM") as psKB:
        kbar32 = [kbp.tile([128, D], F32, name=f"kb{i}") for i in range(NL)]
        kb_ps = [psKB.tile([128, 512], F32, tag=f"kbps{i}", name=f"kbps{i}")
                 for i in range(NL)]
        for jt in range(NJ):
            ct = cntp.tile([128, L], F32R, tag="cnt", name=f"cnt{jt}")
            dmac(ct[:], CntT.ap()[jt * 128:(jt + 1) * 128, :])
            for lt in range(NL):
                nc.tensor.matmul(
                    kb_ps[lt][:],
                    ct[:, lt * 128:(lt + 1) * 128],
                    k_r[jt][:],
                    start=(jt == 0), stop=(jt == NJ - 1))
        for lt in range(NL):
            nc.vector.tensor_copy(kbar32[lt][:], kb_ps[lt][:])
        for lt in range(NL):
            E = escrp.tile([128, D], F32, tag="E", name="E")
            nc.vector.tensor_tensor(out=E[:], in0=q32[lt][:],
                                    in1=kbar32[lt][:], op=OP.mult)
            Ered = escrp.tile([128, H], F32, tag="Ered", name="Ered")
            nc.vector.tensor_reduce(
                out=Ered[:], in_=E[:].rearrange("p (h d) -> p h d", d=DK),
                axis=AX.X, op=OP.add)
            nc.vector.scalar_tensor_tensor(
                out=M_sb[lt][:], in0=Ered[:], scalar=-1.0 / U,
                in1=mt_sb[lt][:], op0=OP.mult, op1=OP.add)
            if dbg:
                dma(DBG_M.ap()[lt * 128:(lt + 1) * 128, :], M_sb[lt][:])
    kr_cm.__exit__(None, None, None)

    # =================== phase 4: topk(M) -> m_top ===================
    psT_cm = tc.tile_pool(name="psT", bufs=1, space="PSUM")
    psT = psT_cm.__enter__()
    if True:
        psMT = psT.tile([H, L], F32, tag="mtT", name="mtT_ps")
        for lt in range(NL):
            nc.tensor.matmul(psMT[:, lt * 128:(lt + 1) * 128],
                             M_sb[lt][:], ident32[:],
                             is_transpose=True, start=True, stop=True)
        mtM_cm.__exit__(None, None, None)
        oaT_cm, oaT_p = pool("oaTp")     # reserved early for LIFO nesting
        oaT = [oaT_p.tile([128, L], F32R, name=f"oa{i}") for i in range(ND)]
        atn_cm, atn = pool("atn")
        MTa = atn.tile([H, L], F32, name="MTa")
        MTb = atn.tile([H, L], F32, name="MTb")
        nc.vector.tensor_copy(MTa[:], psMT[:])
        topv = atn.tile([H, 40], F32, name="topv")
        topi = atn.tile([H, 40], U32, name="topi")
        cur, nxt = MTa, MTb
        for r in range(5):
            vs = topv[:, r * 8:(r + 1) * 8]
            nc.vector.max(vs, cur[:])
            nc.vector.max_index(topi[:, r * 8:(r + 1) * 8], vs, cur[:])
            if r < 4:
                nc.vector.match_replace(nxt[:], vs, cur[:], NEG)
                cur, nxt = nxt, cur
        mtop_i = atn.tile([H, U], I32, name="mtop_i")
        nc.vector.tensor_copy(mtop_i[:], topi[:, 0:U])
        dma(MTOP.ap(), mtop_i[:])
        mtop_f = atn.tile([H, UP], F32, name="mtop_f")
        nc.vector.memset(mtop_f[:], -1.0)
        nc.vector.tensor_copy(mtop_f[:, 0:U], topi[:, 0:U])
        # bounce via DRAM for the [36, 8] transpose and [1, 288] flat row
        dma(mscr.ap().rearrange("x (a b) -> (x a) b", a=H, b=UP), mtop_f[:])
        m_topT = atn.tile([UP, H], F32, name="m_topT")
        dma(m_topT[:], mscr.ap().rearrange("x (a b) -> (x b) a", a=H, b=UP))
        mbc_row = atn.tile([1, H * UP], F32, name="mbc_row")
        dma(mbc_row[:], mscr.ap())
        psbc = psT.tile([128, H * UP], F32, tag="mbc", name="mbc_ps")
        nc.tensor.matmul(psbc[:], ones_r32[:, 0:128], mbc_row[:],
                         start=True, stop=True)
        mbc = atn.tile([128, H * UP], F32, name="mbc")
        nc.vector.tensor_copy(mbc[:], psbc[:])
    psT_cm.__exit__(None, None, None)

    # ============ phase 5: S matrices, q_red, scores, softmax ============
    with tc.tile_pool(name="psS", bufs=2, space="PSUM") as psS, \
         tc.tile_pool(name="psSC", bufs=2, space="PSUM") as psSC:
        S_T = [atn.tile([128, H * UP], F32, name=f"ST{j}") for j in range(NJ)]
        for jt in range(NJ):
            nc.vector.scalar_tensor_tensor(
                out=S_T[jt][:], in0=mbc[:], scalar=float(-jt * 128),
                in1=iotac_sb[:], op0=OP.add, op1=OP.is_equal)
        qred_r = [atn.tile([128, UP], F32R, name=f"qredp{hp}")
                  for hp in range(4)]
        for h in range(H):
            psq = psS.tile([64, UP], F32, tag="qred", name="qred_ps")
            for lt in range(NL):
                nc.tensor.matmul(
                    psq[:],
                    q32[lt][:, h * DK:(h + 1) * DK],
                    S_T[lt][:, h * UP:(h + 1) * UP],
                    start=(lt == 0), stop=(lt == NL - 1))
            nc.scalar.copy(
                qred_r[h // 2][(h % 2) * 64:(h % 2) * 64 + 64, :], psq[:])
        rsum = [atn.tile([UP, 1], F32, name=f"rs{h}") for h in range(H)]
        recip = [atn.tile([UP, 1], F32, name=f"rc{h}") for h in range(H)]
        attn_r = [atn.tile([UP, L], F32R, name=f"at{h}") for h in range(H)]
        for h in range(H):
            pssc = psSC.tile([UP, L], F32, tag="scores", name="sc_ps")
            for l2 in range(2):
                nc.tensor.matmul(
                    pssc[:, l2 * 512:(l2 + 1) * 512],
                    qred_r[h // 2][(h % 2) * 64:(h % 2) * 64 + 64, :],
                    kT_hi[h // 2][(h % 2) * 64:(h % 2) * 64 + 64,
                                  l2 * 512:(l2 + 1) * 512],
                    start=True, stop=True)
            rmax = atn.tile([UP, 1], F32, tag="rmax", name="rmax", bufs=2)
            nc.vector.reduce_max(rmax[:], pssc[:], axis=AX.X)
            nmax = atn.tile([UP, 1], F32, tag="nmax", name="nmax", bufs=2)
            nc.vector.tensor_scalar_mul(nmax[:], rmax[:], -0.125)
            nc.scalar.activation(
                attn_r[h][:], pssc[:], ACTF.Exp,
                bias=nmax[:], scale=0.125, accum_out=rsum[h][:])
            nc.vector.reciprocal(recip[h][:], rsum[h][:])
            nc.vector.tensor_scalar(
                out=attn_r[h][:], in0=attn_r[h][:], scalar1=recip[h][:],
                scalar2=None, op0=OP.mult)
    # ============ phase 5b: attn^T, upd/delta, ctx^T ============
    with tc.tile_pool(name="psAT", bufs=2, space="PSUM") as psAT:
        attnT = [atn.tile([128, H * UP], F32R, name=f"aT{lt}")
                 for lt in range(NL)]
        for lt in range(NL):
            psa = psAT.tile([128, H * UP], F32, tag="aT", name="aT_ps")
            for h in range(H):
                nc.tensor.matmul(
                    psa[:, h * UP:(h + 1) * UP],
                    attn_r[h][:, lt * 128:(lt + 1) * 128],
                    identr[0:UP, 0:UP],
                    start=True, stop=True)
            nc.scalar.copy(attnT[lt][:], psa[:])
        delta_r = atn.tile([UP, H * DV], F32R, name="delta_r")
        for h in range(H):
            psu = psAT.tile([UP, DV], F32, tag="upd", name="upd_ps")
            for lt in range(NL):
                nc.tensor.matmul(
                    psu[:],
                    attnT[lt][:, h * UP:(h + 1) * UP],
                    v_r[lt][:, h * DV:(h + 1) * DV],
                    start=(lt == 0), stop=False)
            nc.tensor.matmul(psu[:], ones_rr[:, 0:UP],
                             negvm_r[:, h * DV:(h + 1) * DV],
                             start=False, stop=True)
            nc.scalar.copy(delta_r[:, h * DV:(h + 1) * DV], psu[:])
        ctxT = [atn.tile([128, L], F32R, name=f"cx{i}") for i in range(ND)]
        srp_cm = tc.tile_pool(name="srp", bufs=2)
        srp = srp_cm.__enter__()
        for h in range(H):
            S_h = srp.tile([UP, L], F32R, tag="Sh", name=f"S{h}")
            nc.vector.tensor_scalar(
                out=S_h[:], in0=iota_row_sb[:],
                scalar1=m_topT[:, h:h + 1], scalar2=None, op0=OP.is_equal)
            for l2 in range(2):
                psc = psAT.tile([64, 512], F32, tag="ctx", name="ctx_ps")
                nc.tensor.matmul(
                    psc[:],
                    delta_r[:, h * DV:(h + 1) * DV],
                    S_h[:, l2 * 512:(l2 + 1) * 512],
                    start=True, stop=False)
                nc.tensor.matmul(
                    psc[:],
                    vmean_r[:, h * DV:(h + 1) * DV],
                    ones_rr[:, 0:512],
                    start=False, stop=True)
                nc.scalar.copy(
                    ctxT[h // 2][(h % 2) * 64:(h % 2) * 64 + 64,
                                 l2 * 512:(l2 + 1) * 512],
                    psc[:])
        srp_cm.__exit__(None, None, None)

    # =================== phase 6: Wo -> attn_out^T ===================
    with tc.tile_pool(name="wo", bufs=1) as wop, \
         tc.tile_pool(name="psWO", bufs=4, space="PSUM") as psWO:
        Wo_r = []
        for i in range(ND):
            w = wop.tile([128, D], F32R, name=f"wo{i}")
            dmac(w[:], Wo.ap()[i * 128:(i + 1) * 128, :])
            Wo_r.append(w)
        for dt_ in range(ND):
            for l2 in range(2):
                pso = psWO.tile([128, 512], F32, tag="oa", name="oa_ps")
                for hc in range(ND):
                    nc.tensor.matmul(
                        pso[:],
                        Wo_r[hc][:, dt_ * 128:(dt_ + 1) * 128],
                        ctxT[hc][:, l2 * 512:(l2 + 1) * 512],
                        start=(hc == 0), stop=(hc == ND - 1))
                nc.scalar.copy(oaT[dt_][:, l2 * 512:(l2 + 1) * 512], pso[:])
                if dbg:
                    dmac(DBG_AO.ap()[dt_ * 128:(dt_ + 1) * 128,
                                     l2 * 512:(l2 + 1) * 512],
                         oaT[dt_][:, l2 * 512:(l2 + 1) * 512])
    atn_cm.__exit__(None, None, None)

    # ============ phase 7: FFN + residual + layernorm ============
    with tc.tile_pool(name="ffn", bufs=1) as ffn, \
         tc.tile_pool(name="hTp", bufs=1) as hTp, \
         tc.tile_pool(name="lnp", bufs=2) as lnp, \
         tc.tile_pool(name="psF", bufs=3, space="PSUM") as psF:
        c1_r = []
        for i in range(ND):
            w = ffn.tile([128, DFF], F32R, name=f"c1_{i}")
            dmac(w[:], c1T.ap()[i * 128:(i + 1) * 128, :])
            c1_r.append(w)
        c2_r = []
        for i in range(NF):
            w = ffn.tile([128, D], F32R, name=f"c2_{i}")
            dmac(w[:], c2T.ap()[i * 128:(i + 1) * 128, :])
            c2_r.append(w)
        for l2 in range(2):
            hT = [hTp.tile([128, 512], F32R, tag=f"hT{i}", name=f"hT{l2}_{i}")
                  for i in range(NF)]
            for ft in range(NF):
                psh = psF.tile([128, 512], F32, tag="h", name="h_ps")
                for dd in range(ND):
                    nc.tensor.matmul(
                        psh[:],
                        c1_r[dd][:, ft * 128:(ft + 1) * 128],
                        oaT[dd][:, l2 * 512:(l2 + 1) * 512],
                        start=(dd == 0), stop=(dd == ND - 1))
                nc.scalar.activation(hT[ft][:], psh[:], ACTF.Gelu)
            for li in range(4):
                lt = l2 * 4 + li
                psz = psF.tile([128, 512], F32, tag="z", name="z_ps")
                for ft in range(NF):
                    nc.tensor.matmul(
                        psz[:],
                        hT[ft][:, li * 128:(li + 1) * 128],
                        c2_r[ft][:],
                        start=(ft == 0), stop=False)
                for dd in range(ND):
                    nc.tensor.matmul(
                        psz[:, dd * 128:(dd + 1) * 128],
                        oaT[dd][:, lt * 128:(lt + 1) * 128],
                        identr[:],
                        start=False, stop=(dd == ND - 1))
                stats = lnp.tile([128, 6], F32, tag="st", name="st")
                aggr = lnp.tile([128, 2], F32, tag="ag", name="ag")
                nc.vector.bn_stats(stats[:], psz[:])
                nc.vector.bn_aggr(aggr[:], stats[:])
                veps = lnp.tile([128, 1], F32, tag="veps", name="veps")
                nc.vector.tensor_scalar_add(veps[:], aggr[:, 1:2], 1e-5)
                std = lnp.tile([128, 1], F32, tag="std", name="std")
                nc.scalar.sqrt(std[:], veps[:])
                rstd = lnp.tile([128, 1], F32, tag="rstd", name="rstd")
                nc.vector.reciprocal(rstd[:], std[:])
                o = lnp.tile([128, D], F32, tag="o", name="o")
                nc.vector.tensor_scalar(
                    out=o[:], in0=psz[:], scalar1=aggr[:, 0:1], scalar2=rstd[:],
                    op0=OP.subtract, op1=OP.mult)
                dma(OUT.ap()[lt * 128:(lt + 1) * 128, :], o[:])
    oaT_cm.__exit__(None, None, None)
    q32_cm.__exit__(None, None, None)
    kThi_cm.__exit__(None, None, None)
    vr_cm.__exit__(None, None, None)
    cst_cm.__exit__(None, None, None)


_BUILD_CACHE = {}


def _build(dbg=False):
    key = ("nc", dbg)
    if key in _BUILD_CACHE:
        return _BUILD_CACHE[key]
    nc = bass.Bass("TRN2", target_bir_lowering=False, debug=False)
    with tile.TileContext(nc, pool_alloc_mode="queue") as tc:
        _emit(nc, tc, dbg=dbg)
    _BUILD_CACHE[key] = nc
    return nc


def _host_prep(inputs):
    """Per-core input maps. Numpy only: layout/index transforms, no math."""
    x = np.ascontiguousarray(np.asarray(inputs["x"], dtype=np.float32))
    idx = np.asarray(inputs["index_sample"]).astype(np.int64)

    for nm in ("bq", "bk", "bv", "bo", "conv1_b", "conv2_b"):
        if nm in inputs and not np.allclose(np.asarray(inputs[nm]), 0.0):
            raise NotImplementedError(f"nonzero bias {nm} not supported")

    cnt = np.zeros((L, L), dtype=np.float32)
    np.add.at(cnt, (np.repeat(np.arange(L), idx.shape[1]), idx.reshape(-1)),
              1.0)
    CntT = np.ascontiguousarray(cnt.T)
    maskadd = np.where(cnt > 0, np.float32(0.0), np.float32(MASKNEG))

    shared = {
        "Wq": np.asarray(inputs["Wq"], dtype=np.float32),
        "Wk": np.asarray(inputs["Wk"], dtype=np.float32),
        "Wv": np.asarray(inputs["Wv"], dtype=np.float32),
        "Wo": np.asarray(inputs["Wo"], dtype=np.float32),
        "c1T": np.ascontiguousarray(
            np.asarray(inputs["conv1_w"], dtype=np.float32).T),
        "c2T": np.ascontiguousarray(
            np.asarray(inputs["conv2_w"], dtype=np.float32).T),
        "CntT": CntT,
        "maskadd": maskadd,
        "iota_row": np.broadcast_to(
            np.arange(L, dtype=np.float32), (UP, L)).copy(),
        "iotac": np.broadcast_to(
            np.arange(128, dtype=np.float32)[:, None], (128, H * UP)).copy(),
        "ident": np.eye(128, dtype=np.float32),
        "onesr": np.ones((1, D), dtype=np.float32),
        "onesc": np.ones((128, 1), dtype=np.float32),
    }
    in_maps = []
    for b in range(B):
        m = dict(shared)
        m["xT"] = np.ascontiguousarray(x[b].T)
        in_maps.append(m)
    return in_maps


def _postprocess(out, gamma, beta):
    g = np.asarray(gamma, dtype=np.float32)
    bta = np.asarray(beta, dtype=np.float32)
    if not np.allclose(g, 1.0) or not np.allclose(bta, 0.0):
        out = out * g + bta
    return out


def kernel(**inputs):
    nc = split_multi_waits(_build(dbg=False))
    in_maps = _host_prep(inputs)
    res = bass_utils.run_bass_kernel_spmd(nc, in_maps, list(range(B)))
    out = np.stack([res.results[b]["OUT"] for b in range(B)])
    out = _postprocess(out, inputs["gamma"], inputs["beta"])
    m_top = np.stack([res.results[b]["MTOP"] for b in range(B)]).astype(np.int32)
    return out, m_top, m_top


# revision 15
# speedup vs baseline: 1.0326x; 1.0326x over previous
"""Trainium2 Bass kernel for nn_EncoderLayer_64321430225657 (ProbSparse encoder layer).

Self-contained: hardcodes shapes/sharding. Data-parallel over batch B=8 across
8 NeuronCores (one batch element per core). All sparse gathers/scatters are
reformulated as dense matmuls using host-precomputed index-derived matrices and
device-built one-hot selection matrices. The M-measure path (which decides the
exact top-35 query indices, part of the output) runs in split-fp32r 3-pass
matmuls for fp32-exact results; everything downstream of the top-k runs 1-pass
fp32r (~1.2e-4 relative error).
"""
import sys
import os

for _p in ("/opt/trn_rl_repo",):
    if _p not in sys.path and os.path.isdir(_p):
        sys.path.insert(0, _p)

import numpy as np
import ml_dtypes

import concourse.bass as bass
import concourse.tile as tile
import concourse.mybir as mybir
from concourse import bass_utils

F32 = mybir.dt.float32
F32R = mybir.dt.float32r
BF16 = mybir.dt.bfloat16
I32 = mybir.dt.int32
U32 = mybir.dt.uint32
AX = mybir.AxisListType
OP = mybir.AluOpType
ACTF = mybir.ActivationFunctionType

B, L, D = 8, 1024, 512
H, DK, DV = 8, 64, 64
U, UP = 35, 36
DFF = 2048
NL, ND, NJ, NF = 8, 4, 8, 16  # 128-chunks of L, D, L(keys), DFF
NEG = -3.0e38
MASKNEG = -1.0e30
GROUPS = ((0, 3), (3, 6), (6, 8))  # head groups for packed softmax/transposes


def split_multi_waits(nc):
    """Walrus codegen in this container supports only one sync-wait per
    instruction; hoist extras onto preceding NOPs on the same engine."""
    for f in nc.m.functions:
        for bb in f.blocks:
            insts = list(bb.instructions)
            new_insts = []
            changed = False
            for ins in insts:
                si = ins.sync_info
                if si is not None and len(si.on_wait) > 1:
                    waits = list(si.on_wait)
                    for k, w in enumerate(waits[:-1]):
                        new_insts.append(mybir.InstNoOp(
                            name=f"{ins.name}-wsplit{k}",
                            sync_info=mybir.SyncInfo(on_wait=[w], on_update=[]),
                            bass_nofuse=True,
                            engine=ins.engine,
                        ))
                    si.on_wait = [waits[-1]]
                    ins.sync_info = si
                    changed = True
                new_insts.append(ins)
            if changed:
                bb.instructions = new_insts
    return nc


def _emit(nc, tc, dbg=False):
    # ---- DRAM I/O ----
    xT = nc.dram_tensor("xT", [D, L], F32, kind="ExternalInput")
    Wq = nc.dram_tensor("Wq", [D, H * DK], F32, kind="ExternalInput")
    Wk = nc.dram_tensor("Wk", [D, H * DK], F32, kind="ExternalInput")
    Wv = nc.dram_tensor("Wv", [D, H * DV], F32R, kind="ExternalInput")
    Wo = nc.dram_tensor("Wo", [H * DV, D], F32R, kind="ExternalInput")
    c1T = nc.dram_tensor("c1T", [D, DFF], F32R, kind="ExternalInput")
    c2T = nc.dram_tensor("c2T", [DFF, D], F32R, kind="ExternalInput")
    CntT = nc.dram_tensor("CntT", [L, L], F32R, kind="ExternalInput")
    maskadd = nc.dram_tensor("maskadd", [L, L], F32R, kind="ExternalInput")
    iota_row = nc.dram_tensor("iota_row", [UP, L], F32, kind="ExternalInput")
    iotac = nc.dram_tensor("iotac", [128, H * UP], F32, kind="ExternalInput")
    ident = nc.dram_tensor("ident", [128, 128], F32, kind="ExternalInput")
    identrd = nc.dram_tensor("identrd", [128, 128], F32R, kind="ExternalInput")
    onesrd = nc.dram_tensor("onesrd", [1, D], F32R, kind="ExternalInput")
    onescd = nc.dram_tensor("onescd", [128, 1], F32R, kind="ExternalInput")
    onesr = nc.dram_tensor("onesr", [1, D], F32, kind="ExternalInput")
    onesc = nc.dram_tensor("onesc", [128, 1], F32, kind="ExternalInput")

    OUT = nc.dram_tensor("OUT", [L, D], F32, kind="ExternalOutput")
    MTOP = nc.dram_tensor("MTOP", [H, U], I32, kind="ExternalOutput")
    mscr = nc.dram_tensor("mtop_scratch", [1, H * UP], F32)  # internal bounce
    if dbg:
        DBG_M = nc.dram_tensor("DBG_M", [L, H], F32, kind="ExternalOutput")
        DBG_QT = nc.dram_tensor("DBG_QT", [D, L], F32, kind="ExternalOutput")
        DBG_AO = nc.dram_tensor("DBG_AO", [D, L], F32, kind="ExternalOutput")

    dma = nc.sync.dma_start
    dmac = nc.gpsimd.dma_start  # dtype-casting DMAs (fp32 -> fp32r)

    def pool(name, bufs=1):
        cm = tc.tile_pool(name=name, bufs=bufs)
        return cm, cm.__enter__()

    # =================== constants (live forever) ===================
    cst_cm, cst = pool("cst")
    ident32 = cst.tile([128, 128], F32, name="ident32")
    dma(ident32[:], ident.ap())
    identr = cst.tile([128, 128], F32R, name="identr")
    dma(identr[:], identrd.ap())
    ones_r32 = cst.tile([1, D], F32, name="ones_r32")
    dma(ones_r32[:], onesr.ap())
    ones_rr = cst.tile([1, D], F32R, name="ones_rr")
    dma(ones_rr[:], onesrd.ap())
    ones_cr = cst.tile([128, 1], F32R, name="ones_cr")
    dma(ones_cr[:], onescd.ap())
    iota_row_sb = cst.tile([UP, L], F32, name="iota_row_sb")
    dma(iota_row_sb[:], iota_row.ap())
    iotac_sb = cst.tile([128, H * UP], F32, name="iotac_sb")
    dma(iotac_sb[:], iotac.ap())

    # ---- mid-life tensor pools (creation order = reverse release order) ----
    vr_cm, vr_p = pool("vrp")            # released after phase 5b
    v_r = [vr_p.tile([128, D], F32R, name=f"vr{i}") for i in range(NL)]
    vmean_r = vr_p.tile([1, D], F32R, name="vmean_r")
    negvm_r = vr_p.tile([1, D], F32R, name="negvm_r")
    kThi_cm, kThi_p = pool("kThi")       # released after phase 5
    kT_hi = [kThi_p.tile([128, L], F32R, name=f"kThi{i}") for i in range(ND)]
    q32_cm, q32_p = pool("q32p")         # released after phase 5
    q32 = [q32_p.tile([128, D], F32, name=f"q32_{i}") for i in range(NL)]
    mtM_cm, mtM_p = pool("mtM")          # released in phase 4
    mt_sb = [mtM_p.tile([128, H], F32, name=f"mt{i}") for i in range(NL)]
    M_sb = [mtM_p.tile([128, H], F32, name=f"M{i}") for i in range(NL)]
    kr_cm, kr_p = pool("krp")            # released after phase 3
    k_r = [kr_p.tile([128, D], F32R, name=f"kr{i}") for i in range(NL)]
    k_lo = [kr_p.tile([128, D], F32R, name=f"klo{i}") for i in range(NL)]
    qkx_cm, qkx = pool("qkx")            # released after phase 2
    qT_hi = [qkx.tile([128, L], F32R, name=f"qThi{i}") for i in range(ND)]
    qT_lo = [qkx.tile([128, L], F32R, name=f"qTlo{i}") for i in range(ND)]
    kT_lo = [qkx.tile([128, L], F32R, name=f"kTlo{i}") for i in range(ND)]

    # =================== phase 1: splits + projections ===================
    with tc.tile_pool(name="p1", bufs=1) as p1, \
         tc.tile_pool(name="tmp1", bufs=1) as tmp1, \
         tc.tile_pool(name="ps1", bufs=4, space="PSUM") as ps1, \
         tc.tile_pool(name="psA", bufs=1, space="PSUM") as psA:
        x_hi = [p1.tile([128, L], F32R, name=f"xh{i}") for i in range(ND)]
        x_lo = [p1.tile([128, L], F32R, name=f"xl{i}") for i in range(ND)]
        for i in range(ND):
            t32 = tmp1.tile([128, L], F32, tag="t32x", name=f"xT32_{i}")
            dma(t32[:], xT.ap()[i * 128:(i + 1) * 128, :])
            nc.scalar.copy(x_hi[i][:], t32[:])
            nc.vector.tensor_tensor(
                out=x_lo[i][:], in0=t32[:], in1=x_hi[i][:], op=OP.subtract)

        for nm, Wd, t_hi, t_lo, sink, cvt in (
                ("q", Wq, qT_hi, qT_lo, q32, "dve32"),
                ("k", Wk, kT_hi, kT_lo, k_r, "actr")):
            with tc.tile_pool(name=f"w{nm}", bufs=1) as wp:
                w_hi, w_lo = [], []
                for i in range(ND):
                    w32 = tmp1.tile([128, D], F32, tag="t32w",
                                    name=f"w32_{nm}{i}")
                    dma(w32[:], Wd.ap()[i * 128:(i + 1) * 128, :])
                    hi = wp.tile([128, D], F32R, name=f"whi_{nm}{i}")
                    lo = wp.tile([128, D], F32R, name=f"wlo_{nm}{i}")
                    nc.scalar.copy(hi[:], w32[:])
                    nc.vector.tensor_tensor(out=lo[:], in0=w32[:], in1=hi[:],
                                            op=OP.subtract)
                    w_hi.append(hi)
                    w_lo.append(lo)
                # transposed layout [hd, l]: 3-pass exact -> hi/lo splits
                for dk in range(ND):
                    for l2 in range(2):
                        ps = ps1.tile([128, 512], F32, tag="proj",
                                      name="proj_ps")
                        nmm = 0
                        for dd in range(ND):
                            for wset, xset in ((w_hi, x_hi), (w_hi, x_lo),
                                               (w_lo, x_hi)):
                                nc.tensor.matmul(
                                    ps[:],
                                    wset[dd][:, dk * 128:(dk + 1) * 128],
                                    xset[dd][:, l2 * 512:(l2 + 1) * 512],
                                    start=(nmm == 0), stop=(nmm == 11))
                                nmm += 1
                        hi_sl = t_hi[dk][:, l2 * 512:(l2 + 1) * 512]
                        lo_sl = t_lo[dk][:, l2 * 512:(l2 + 1) * 512]
                        nc.scalar.copy(hi_sl, ps[:])
                        nc.vector.tensor_tensor(out=lo_sl, in0=ps[:],
                                                in1=hi_sl, op=OP.subtract)
                # normal layout [l, hd]: 3-pass exact
                for lt in range(NL):
                    ps = ps1.tile([128, 512], F32, tag="proj", name="qkn_ps")
                    nmm = 0
                    for dd in range(ND):
                        for wset, xset in ((w_hi, x_hi), (w_hi, x_lo),
                                           (w_lo, x_hi)):
                            nc.tensor.matmul(
                                ps[:],
                                xset[dd][:, lt * 128:(lt + 1) * 128],
                                wset[dd][:],
                                start=(nmm == 0), stop=(nmm == 11))
                            nmm += 1
                    if cvt == "dve32":
                        nc.vector.tensor_copy(sink[lt][:], ps[:])
                    else:
                        nc.scalar.copy(sink[lt][:], ps[:])
                        nc.vector.tensor_tensor(
                            out=k_lo[lt][:], in0=ps[:], in1=sink[lt][:],
                            op=OP.subtract)
        if dbg:
            for dk in range(ND):
                dmac(DBG_QT.ap()[dk * 128:(dk + 1) * 128, :], qT_hi[dk][:])

        # v: 1-pass fp32r, normal layout [l, hd]
        with tc.tile_pool(name="wv", bufs=1) as wvp:
            Wv_r = []
            for i in range(ND):
                wv = wvp.tile([128, D], F32R, name=f"wv{i}")
                dma(wv[:], Wv.ap()[i * 128:(i + 1) * 128, :])
                Wv_r.append(wv)
            for lt in range(NL):
                ps = ps1.tile([128, 512], F32, tag="proj", name="v_ps")
                for dd in range(ND):
                    nc.tensor.matmul(
                        ps[:],
                        x_hi[dd][:, lt * 128:(lt + 1) * 128],
                        Wv_r[dd][:],
                        start=(dd == 0), stop=(dd == ND - 1))
                nc.scalar.copy(v_r[lt][:], ps[:])

        # vmean row [1, 512] and its negation (fp32r rows for K=1 matmuls)
        psv = psA.tile([1, 512], F32, tag="vm", name="vm_ps")
        for lt in range(NL):
            nc.tensor.matmul(psv[:], ones_cr[:], v_r[lt][:],
                             start=(lt == 0), stop=(lt == NL - 1))
        nc.scalar.mul(vmean_r[:], psv[:], 1.0 / L)
        nc.scalar.mul(negvm_r[:], psv[:], -1.0 / L)

    # =================== phase 2: QK + masked max ===================
    # The sampling mask (0 / -1e30) is injected into PSUM via an identity
    # matmul as the first accumulation step; a single reduce_max per
    # (head, l-tile) then yields the masked max.
    with tc.tile_pool(name="mask", bufs=2) as maskp, \
         tc.tile_pool(name="psQK", bufs=3, space="PSUM") as psQK:
        for lt in range(NL):
            mtile = maskp.tile([128, L], F32R, tag="mask", name=f"mask{lt}")
            dma(mtile[:], maskadd.ap()[lt * 128:(lt + 1) * 128, :])
            lsl = slice(lt * 128, (lt + 1) * 128)
            for hp in range(4):  # heads (2hp, 2hp+1) live in qT tile hp
                for hh in range(2):
                    h = 2 * hp + hh
                    rsl = slice(hh * 64, hh * 64 + 64)
                    tp = (hh * 64, 0)
                    psh = psQK.tile([128, L], F32, tag="qk", name=f"qk{h}")
                    for j2 in range(2):
                        jsl = slice(j2 * 512, (j2 + 1) * 512)
                        nc.tensor.matmul(
                            psh[:, jsl], identr[:], mtile[:, jsl],
                            start=True, stop=False)
                        for pa, pb in ((qT_hi, kT_hi), (qT_hi, kT_lo),
                                       (qT_lo, kT_hi)):
                            nc.tensor.matmul(
                                psh[:, jsl],
                                pa[hp][rsl, lsl],
                                pb[hp][rsl, jsl],
                                start=False, stop=(pb is kT_hi and pa is qT_lo),
                                tile_position=tp)
                    nc.vector.reduce_max(
                        mt_sb[lt][:, h:h + 1], psh[:], axis=AX.X)
    qkx_cm.__exit__(None, None, None)

    # =================== phase 3: kbar + mean -> M ===================
    with tc.tile_pool(name="cnt", bufs=2) as cntp, \
         tc.tile_pool(name="kbp", bufs=1) as kbp, \
         tc.tile_pool(name="escr", bufs=2) as escrp, \
         tc.tile_pool(name="psKB", bufs=1, space="PSUM") as psKB:
        kbar32 = [kbp.tile([128, D], F32, name=f"kb{i}") for i in range(NL)]
        kb_ps = [psKB.tile([128, 512], F32, tag=f"kbps{i}", name=f"kbps{i}")
                 for i in range(NL)]
        for jt in range(NJ):
            ct = cntp.tile([128, L], F32R, tag="cnt", name=f"cnt{jt}")
            dma(ct[:], CntT.ap()[jt * 128:(jt + 1) * 128, :])
            for lt in range(NL):
                nc.tensor.matmul(
                    kb_ps[lt][:],
                    ct[:, lt * 128:(lt + 1) * 128],
                    k_r[jt][:],
                    start=(jt == 0), stop=False)
                nc.tensor.matmul(
                    kb_ps[lt][:],
                    ct[:, lt * 128:(lt + 1) * 128],
                    k_lo[jt][:],
                    start=False, stop=(jt == NJ - 1))
        for lt in range(NL):
            nc.vector.tensor_copy(kbar32[lt][:], kb_ps[lt][:])
        for lt in range(NL):
            E = escrp.tile([128, D], F32, tag="E", name="E")
            nc.vector.tensor_tensor(out=E[:], in0=q32[lt][:],
                                    in1=kbar32[lt][:], op=OP.mult)
            Ered = escrp.tile([128, H], F32, tag="Ered", name="Ered")
            nc.vector.tensor_reduce(
                out=Ered[:], in_=E[:].rearrange("p (h d) -> p h d", d=DK),
                axis=AX.X, op=OP.add)
            nc.vector.scalar_tensor_tensor(
                out=M_sb[lt][:], in0=Ered[:], scalar=-1.0 / U,
                in1=mt_sb[lt][:], op0=OP.mult, op1=OP.add)
            if dbg:
                dma(DBG_M.ap()[lt * 128:(lt + 1) * 128, :], M_sb[lt][:])
    kr_cm.__exit__(None, None, None)

    # =================== phase 4: topk(M) -> m_top ===================
    psT_cm = tc.tile_pool(name="psT", bufs=1, space="PSUM")
    psT = psT_cm.__enter__()
    if True:
        psMT = psT.tile([H, L], F32, tag="mtT", name="mtT_ps")
        for lt in range(NL):
            nc.tensor.matmul(psMT[:, lt * 128:(lt + 1) * 128],
                             M_sb[lt][:], ident32[:],
                             is_transpose=True, start=True, stop=True)
        mtM_cm.__exit__(None, None, None)
        oaT_cm, oaT_p = pool("oaTp")     # reserved early for LIFO nesting
        oaT = [oaT_p.tile([128, L], F32R, name=f"oa{i}") for i in range(ND)]
        atn_cm, atn = pool("atn")
        MTa = atn.tile([H, L], F32, name="MTa")
        MTb = atn.tile([H, L], F32, name="MTb")
        nc.vector.tensor_copy(MTa[:], psMT[:])
        topv = atn.tile([H, 40], F32, name="topv")
        topi = atn.tile([H, 40], U32, name="topi")
        cur, nxt = MTa, MTb
        for r in range(5):
            vs = topv[:, r * 8:(r + 1) * 8]
            nc.vector.max(vs, cur[:])
            nc.vector.max_index(topi[:, r * 8:(r + 1) * 8], vs, cur[:])
            if r < 4:
                nc.vector.match_replace(nxt[:], vs, cur[:], NEG)
                cur, nxt = nxt, cur
        mtop_i = atn.tile([H, U], I32, name="mtop_i")
        nc.vector.tensor_copy(mtop_i[:], topi[:, 0:U])
        dma(MTOP.ap(), mtop_i[:])
        mtop_f = atn.tile([H, UP], F32, name="mtop_f")
        nc.vector.memset(mtop_f[:], -1.0)
        nc.vector.tensor_copy(mtop_f[:, 0:U], topi[:, 0:U])
        # bounce via DRAM for the [36, 8] transpose and [1, 288] flat row
        dma(mscr.ap().rearrange("x (a b) -> (x a) b", a=H, b=UP), mtop_f[:])
        m_topT = atn.tile([UP, H], F32, name="m_topT")
        dma(m_topT[:], mscr.ap().rearrange("x (a b) -> (x b) a", a=H, b=UP))
        mbc_row = atn.tile([1, H * UP], F32, name="mbc_row")
        dma(mbc_row[:], mscr.ap())
        psbc = psT.tile([128, H * UP], F32, tag="mbc", name="mbc_ps")
        nc.tensor.matmul(psbc[:], ones_r32[:, 0:128], mbc_row[:],
                         start=True, stop=True)
        mbc = atn.tile([128, H * UP], F32, name="mbc")
        nc.vector.tensor_copy(mbc[:], psbc[:])
    psT_cm.__exit__(None, None, None)

    # ============ phase 5: S matrices, q_red, scores, softmax ============
    with tc.tile_pool(name="psS", bufs=2, space="PSUM") as psS, \
         tc.tile_pool(name="psSC", bufs=2, space="PSUM") as psSC:
        S_T = [atn.tile([128, H * UP], F32, name=f"ST{j}") for j in range(NJ)]
        for jt in range(NJ):
            nc.vector.scalar_tensor_tensor(
                out=S_T[jt][:], in0=mbc[:], scalar=float(-jt * 128),
                in1=iotac_sb[:], op0=OP.add, op1=OP.is_equal)
        qred_r = [atn.tile([128, UP], F32R, name=f"qredp{hp}")
                  for hp in range(4)]
        for h in range(H):
            psq = psS.tile([64, UP], F32, tag="qred", name="qred_ps")
            for lt in range(NL):
                nc.tensor.matmul(
                    psq[:],
                    q32[lt][:, h * DK:(h + 1) * DK],
                    S_T[lt][:, h * UP:(h + 1) * UP],
                    start=(lt == 0), stop=(lt == NL - 1))
            nc.scalar.copy(
                qred_r[h // 2][(h % 2) * 64:(h % 2) * 64 + 64, :], psq[:])
        rsum = [atn.tile([UP, 1], F32, name=f"rs{h}") for h in range(H)]
        recip = [atn.tile([UP, 1], F32, name=f"rc{h}") for h in range(H)]
        attn_r = [atn.tile([UP, L], F32R, name=f"at{h}") for h in range(H)]
        for h in range(H):
            pssc = psSC.tile([UP, L], F32, tag="scores", name="sc_ps")
            for l2 in range(2):
                nc.tensor.matmul(
                    pssc[:, l2 * 512:(l2 + 1) * 512],
                    qred_r[h // 2][(h % 2) * 64:(h % 2) * 64 + 64, :],
                    kT_hi[h // 2][(h % 2) * 64:(h % 2) * 64 + 64,
                                  l2 * 512:(l2 + 1) * 512],
                    start=True, stop=True)
            rmax = atn.tile([UP, 1], F32, tag="rmax", name="rmax", bufs=2)
            nc.vector.reduce_max(rmax[:], pssc[:], axis=AX.X)
            nmax = atn.tile([UP, 1], F32, tag="nmax", name="nmax", bufs=2)
            nc.vector.tensor_scalar_mul(nmax[:], rmax[:], -0.125)
            nc.scalar.activation(
                attn_r[h][:], pssc[:], ACTF.Exp,
                bias=nmax[:], scale=0.125, accum_out=rsum[h][:])
            nc.vector.reciprocal(recip[h][:], rsum[h][:])
            nc.vector.tensor_scalar(
                out=attn_r[h][:], in0=attn_r[h][:], scalar1=recip[h][:],
                scalar2=None, op0=OP.mult)
    # ============ phase 5b: attn^T, upd/delta, ctx^T ============
    with tc.tile_pool(name="psAT", bufs=2, space="PSUM") as psAT:
        attnT = [atn.tile([128, H * UP], F32R, name=f"aT{lt}")
                 for lt in range(NL)]
        for lt in range(NL):
            psa = psAT.tile([128, H * UP], F32, tag="aT", name="aT_ps")
            for h in range(H):
                nc.tensor.matmul(
                    psa[:, h * UP:(h + 1) * UP],
                    attn_r[h][:, lt * 128:(lt + 1) * 128],
                    identr[0:UP, 0:UP],
                    start=True, stop=True)
            nc.scalar.copy(attnT[lt][:], psa[:])
        delta_r = atn.tile([UP, H * DV], F32R, name="delta_r")
        for h in range(H):
            psu = psAT.tile([UP, DV], F32, tag="upd", name="upd_ps")
            for lt in range(NL):
                nc.tensor.matmul(
                    psu[:],
                    attnT[lt][:, h * UP:(h + 1) * UP],
                    v_r[lt][:, h * DV:(h + 1) * DV],
                    start=(lt == 0), stop=False)
            nc.tensor.matmul(psu[:], ones_rr[:, 0:UP],
                             negvm_r[:, h * DV:(h + 1) * DV],
                             start=False, stop=True)
            nc.scalar.copy(delta_r[:, h * DV:(h + 1) * DV], psu[:])
        ctxT = [atn.tile([128, L], F32R, name=f"cx{i}") for i in range(ND)]
        srp_cm = tc.tile_pool(name="srp", bufs=2)
        srp = srp_cm.__enter__()
        for h in range(H):
            S_h = srp.tile([UP, L], F32R, tag="Sh", name=f"S{h}")
            nc.vector.tensor_scalar(
                out=S_h[:], in0=iota_row_sb[:],
                scalar1=m_topT[:, h:h + 1], scalar2=None, op0=OP.is_equal)
            for l2 in range(2):
                psc = psAT.tile([64, 512], F32, tag="ctx", name="ctx_ps")
                nc.tensor.matmul(
                    psc[:],
                    delta_r[:, h * DV:(h + 1) * DV],
                    S_h[:, l2 * 512:(l2 + 1) * 512],
                    start=True, stop=False)
                nc.tensor.matmul(
                    psc[:],
                    vmean_r[:, h * DV:(h + 1) * DV],
                    ones_rr[:, 0:512],
                    start=False, stop=True)
                nc.scalar.copy(
                    ctxT[h // 2][(h % 2) * 64:(h % 2) * 64 + 64,
                                 l2 * 512:(l2 + 1) * 512],
                    psc[:])
        srp_cm.__exit__(None, None, None)

    # =================== phase 6: Wo -> attn_out^T ===================
    with tc.tile_pool(name="wo", bufs=1) as wop, \
         tc.tile_pool(name="psWO", bufs=4, space="PSUM") as psWO:
        Wo_r = []
        for i in range(ND):
            w = wop.tile([128, D], F32R, name=f"wo{i}")
            dma(w[:], Wo.ap()[i * 128:(i + 1) * 128, :])
            Wo_r.append(w)
        for dt_ in range(ND):
            for l2 in range(2):
                pso = psWO.tile([128, 512], F32, tag="oa", name="oa_ps")
                for hc in range(ND):
                    nc.tensor.matmul(
                        pso[:],
                        Wo_r[hc][:, dt_ * 128:(dt_ + 1) * 128],
                        ctxT[hc][:, l2 * 512:(l2 + 1) * 512],
                        start=(hc == 0), stop=(hc == ND - 1))
                nc.scalar.copy(oaT[dt_][:, l2 * 512:(l2 + 1) * 512], pso[:])
                if dbg:
                    dmac(DBG_AO.ap()[dt_ * 128:(dt_ + 1) * 128,
                                     l2 * 512:(l2 + 1) * 512],
                         oaT[dt_][:, l2 * 512:(l2 + 1) * 512])
    atn_cm.__exit__(None, None, None)

    # ============ phase 7: FFN + residual + layernorm ============
    with tc.tile_pool(name="ffn", bufs=1) as ffn, \
         tc.tile_pool(name="hTp", bufs=1) as hTp, \
         tc.tile_pool(name="lnp", bufs=2) as lnp, \
         tc.tile_pool(name="psF", bufs=3, space="PSUM") as psF:
        c1_r = []
        for i in range(ND):
            w = ffn.tile([128, DFF], F32R, name=f"c1_{i}")
            dma(w[:], c1T.ap()[i * 128:(i + 1) * 128, :])
            c1_r.append(w)
        c2_r = []
        for i in range(NF):
            w = ffn.tile([128, D], F32R, name=f"c2_{i}")
            dma(w[:], c2T.ap()[i * 128:(i + 1) * 128, :])
            c2_r.append(w)
        for l2 in range(2):
            hT = [hTp.tile([128, 512], F32R, tag=f"hT{i}", name=f"hT{l2}_{i}")
                  for i in range(NF)]
            for ft in range(NF):
                psh = psF.tile([128, 512], F32, tag="h", name="h_ps")
                for dd in range(ND):
                    nc.tensor.matmul(
                        psh[:],
                        c1_r[dd][:, ft * 128:(ft + 1) * 128],
                        oaT[dd][:, l2 * 512:(l2 + 1) * 512],
                        start=(dd == 0), stop=(dd == ND - 1))
                nc.scalar.activation(hT[ft][:], psh[:], ACTF.Gelu)
            for li in range(4):
                lt = l2 * 4 + li
                psz = psF.tile([128, 512], F32, tag="z", name="z_ps")
                for ft in range(NF):
                    nc.tensor.matmul(
                        psz[:],
                        hT[ft][:, li * 128:(li + 1) * 128],
                        c2_r[ft][:],
                        start=(ft == 0), stop=False)
                for dd in range(ND):
                    nc.tensor.matmul(
                        psz[:, dd * 128:(dd + 1) * 128],
                        oaT[dd][:, lt * 128:(lt + 1) * 128],
                        identr[:],
                        start=False, stop=(dd == ND - 1))
                stats = lnp.tile([128, 6], F32, tag="st", name="st")
                aggr = lnp.tile([128, 2], F32, tag="ag", name="ag")
                nc.vector.bn_stats(stats[:], psz[:])
                nc.vector.bn_aggr(aggr[:], stats[:])
                veps = lnp.tile([128, 1], F32, tag="veps", name="veps")
                nc.vector.tensor_scalar_add(veps[:], aggr[:, 1:2], 1e-5)
                std = lnp.tile([128, 1], F32, tag="std", name="std")
                nc.scalar.sqrt(std[:], veps[:])
                rstd = lnp.tile([128, 1], F32, tag="rstd", name="rstd")
                nc.vector.reciprocal(rstd[:], std[:])
                o = lnp.tile([128, D], F32, tag="o", name="o")
                nc.vector.tensor_scalar(
                    out=o[:], in0=psz[:], scalar1=aggr[:, 0:1], scalar2=rstd[:],
                    op0=OP.subtract, op1=OP.mult)
                dma(OUT.ap()[lt * 128:(lt + 1) * 128, :], o[:])
    oaT_cm.__exit__(None, None, None)
    q32_cm.__exit__(None, None, None)
    kThi_cm.__exit__(None, None, None)
    vr_cm.__exit__(None, None, None)
    cst_cm.__exit__(None, None, None)


_BUILD_CACHE = {}


def _build(dbg=False):
    key = ("nc", dbg)
    if key in _BUILD_CACHE:
        return _BUILD_CACHE[key]
    nc = bass.Bass("TRN2", target_bir_lowering=False, debug=False)
    with tile.TileContext(nc, pool_alloc_mode="queue") as tc:
        _emit(nc, tc, dbg=dbg)
    _BUILD_CACHE[key] = nc
    return nc


def _host_prep(inputs):
    """Per-core input maps. Numpy only: layout/index transforms, no math."""
    x = np.ascontiguousarray(np.asarray(inputs["x"], dtype=np.float32))
    idx = np.asarray(inputs["index_sample"]).astype(np.int64)

    for nm in ("bq", "bk", "bv", "bo", "conv1_b", "conv2_b"):
        if nm in inputs and not np.allclose(np.asarray(inputs[nm]), 0.0):
            raise NotImplementedError(f"nonzero bias {nm} not supported")

    cnt = np.zeros((L, L), dtype=np.float32)
    np.add.at(cnt, (np.repeat(np.arange(L), idx.shape[1]), idx.reshape(-1)),
              1.0)
    CntT = np.ascontiguousarray(cnt.T)
    maskadd = np.where(cnt > 0, np.float32(0.0), np.float32(MASKNEG))

    shared = {
        "Wq": np.asarray(inputs["Wq"], dtype=np.float32),
        "Wk": np.asarray(inputs["Wk"], dtype=np.float32),
        "Wv": np.asarray(inputs["Wv"], dtype=np.float32),
        "Wo": np.asarray(inputs["Wo"], dtype=np.float32),
        "c1T": np.ascontiguousarray(
            np.asarray(inputs["conv1_w"], dtype=np.float32).T),
        "c2T": np.ascontiguousarray(
            np.asarray(inputs["conv2_w"], dtype=np.float32).T),
        "CntT": CntT,
        "maskadd": maskadd,
        "iota_row": np.broadcast_to(
            np.arange(L, dtype=np.float32), (UP, L)).copy(),
        "iotac": np.broadcast_to(
            np.arange(128, dtype=np.float32)[:, None], (128, H * UP)).copy(),
        "ident": np.eye(128, dtype=np.float32),
        "identrd": np.eye(128, dtype=np.float32),
        "onesr": np.ones((1, D), dtype=np.float32),
        "onesrd": np.ones((1, D), dtype=np.float32),
        "onesc": np.ones((128, 1), dtype=np.float32),
        "onescd": np.ones((128, 1), dtype=np.float32),
    }
    in_maps = []
    for b in range(B):
        m = dict(shared)
        m["xT"] = np.ascontiguousarray(x[b].T)
        in_maps.append(m)
    return in_maps


def _postprocess(out, gamma, beta):
    g = np.asarray(gamma, dtype=np.float32)
    bta = np.asarray(beta, dtype=np.float32)
    if not np.allclose(g, 1.0) or not np.allclose(bta, 0.0):
        out = out * g + bta
    return out


def kernel(**inputs):
    nc = split_multi_waits(_build(dbg=False))
    in_maps = _host_prep(inputs)
    res = bass_utils.run_bass_kernel_spmd(nc, in_maps, list(range(B)))
    out = np.stack([res.results[b]["OUT"] for b in range(B)])
    out = _postprocess(out, inputs["gamma"], inputs["beta"])
    m_top = np.stack([res.results[b]["MTOP"] for b in range(B)]).astype(np.int32)
    return out, m_top, m_top


# revision 21
# speedup vs baseline: 1.0469x; 1.0139x over previous
"""Trainium2 Bass kernel for nn_EncoderLayer_64321430225657 (ProbSparse encoder layer).

Self-contained: hardcodes shapes/sharding. Data-parallel over batch B=8 across
8 NeuronCores (one batch element per core). All sparse gathers/scatters are
reformulated as dense matmuls using host-precomputed index-derived matrices and
device-built one-hot selection matrices. The M-measure path (which decides the
exact top-35 query indices, part of the output) runs in split-fp32r 3-pass
matmuls for fp32-exact results; everything downstream of the top-k runs 1-pass
fp32r (~1.2e-4 relative error).
"""
import sys
import os

for _p in ("/opt/trn_rl_repo",):
    if _p not in sys.path and os.path.isdir(_p):
        sys.path.insert(0, _p)

import numpy as np
import ml_dtypes

import concourse.bass as bass
import concourse.tile as tile
import concourse.mybir as mybir
from concourse import bass_utils

F32 = mybir.dt.float32
F32R = mybir.dt.float32r
BF16 = mybir.dt.bfloat16
I32 = mybir.dt.int32
U32 = mybir.dt.uint32
AX = mybir.AxisListType
OP = mybir.AluOpType
ACTF = mybir.ActivationFunctionType

B, L, D = 8, 1024, 512
H, DK, DV = 8, 64, 64
U, UP = 35, 36
DFF = 2048
NL, ND, NJ, NF = 8, 4, 8, 16  # 128-chunks of L, D, L(keys), DFF
NEG = -3.0e38
MASKNEG = -1.0e30
GROUPS = ((0, 3), (3, 6), (6, 8))  # head groups for packed softmax/transposes


def split_multi_waits(nc):
    """Walrus codegen in this container supports only one sync-wait per
    instruction; hoist extras onto preceding NOPs on the same engine."""
    for f in nc.m.functions:
        for bb in f.blocks:
            insts = list(bb.instructions)
            new_insts = []
            changed = False
            for ins in insts:
                si = ins.sync_info
                if si is not None and len(si.on_wait) > 1:
                    waits = list(si.on_wait)
                    for k, w in enumerate(waits[:-1]):
                        new_insts.append(mybir.InstNoOp(
                            name=f"{ins.name}-wsplit{k}",
                            sync_info=mybir.SyncInfo(on_wait=[w], on_update=[]),
                            bass_nofuse=True,
                            engine=ins.engine,
                        ))
                    si.on_wait = [waits[-1]]
                    ins.sync_info = si
                    changed = True
                new_insts.append(ins)
            if changed:
                bb.instructions = new_insts
    return nc


def _emit(nc, tc, dbg=False, stages=99):
    # ---- DRAM I/O ----
    xT = nc.dram_tensor("xT", [D, L], F32, kind="ExternalInput")
    Wq = nc.dram_tensor("Wq", [D, H * DK], F32, kind="ExternalInput")
    Wk = nc.dram_tensor("Wk", [D, H * DK], F32, kind="ExternalInput")
    Wv = nc.dram_tensor("Wv", [D, H * DV], F32R, kind="ExternalInput")
    Wo = nc.dram_tensor("Wo", [H * DV, D], F32R, kind="ExternalInput")
    c1T = nc.dram_tensor("c1T", [D, DFF], F32R, kind="ExternalInput")
    c2T = nc.dram_tensor("c2T", [DFF, D], F32R, kind="ExternalInput")
    CntT = nc.dram_tensor("CntT", [L, L], F32, kind="ExternalInput")
    maskadd = nc.dram_tensor("maskadd", [L, L], F32R, kind="ExternalInput")
    iota_row = nc.dram_tensor("iota_row", [UP, L], F32, kind="ExternalInput")
    iotac = nc.dram_tensor("iotac", [128, H * UP], F32, kind="ExternalInput")
    ident = nc.dram_tensor("ident", [128, 128], F32, kind="ExternalInput")
    identrd = nc.dram_tensor("identrd", [128, 128], F32R, kind="ExternalInput")
    onesrd = nc.dram_tensor("onesrd", [1, D], F32R, kind="ExternalInput")
    onescd = nc.dram_tensor("onescd", [128, 1], F32R, kind="ExternalInput")
    onesr = nc.dram_tensor("onesr", [1, D], F32, kind="ExternalInput")
    onesc = nc.dram_tensor("onesc", [128, 1], F32, kind="ExternalInput")

    OUT = nc.dram_tensor("OUT", [L, D], F32, kind="ExternalOutput")
    MTOP = nc.dram_tensor("MTOP", [H, U], I32, kind="ExternalOutput")
    mscr = nc.dram_tensor("mtop_scratch", [1, H * UP], F32)  # internal bounce
    if dbg:
        DBG_M = nc.dram_tensor("DBG_M", [L, H], F32, kind="ExternalOutput")
        DBG_MT2 = nc.dram_tensor("DBG_MT2", [L, H], F32, kind="ExternalOutput")
        DBG_QLO = nc.dram_tensor("DBG_QLO", [D, L], F32R, kind="ExternalOutput")
        DBG_XH = nc.dram_tensor("DBG_XH", [D, L], F32R, kind="ExternalOutput")
        DBG_XL = nc.dram_tensor("DBG_XL", [D, L], F32R, kind="ExternalOutput")
        DBG_WH = nc.dram_tensor("DBG_WH", [D, D], F32R, kind="ExternalOutput")
        DBG_WL = nc.dram_tensor("DBG_WL", [D, D], F32R, kind="ExternalOutput")
        DBG_KHI = nc.dram_tensor("DBG_KHI", [D, L], F32R, kind="ExternalOutput")
        DBG_KLO = nc.dram_tensor("DBG_KLO", [D, L], F32R, kind="ExternalOutput")
        DBG_QT = nc.dram_tensor("DBG_QT", [D, L], F32, kind="ExternalOutput")
        DBG_AO = nc.dram_tensor("DBG_AO", [D, L], F32, kind="ExternalOutput")

    dma = nc.sync.dma_start
    dmac = nc.gpsimd.dma_start  # dtype-casting DMAs (fp32 -> fp32r)

    def pool(name, bufs=1):
        cm = tc.tile_pool(name=name, bufs=bufs)
        return cm, cm.__enter__()

    # =================== constants (live forever) ===================
    cst_cm, cst = pool("cst")
    ident32 = cst.tile([128, 128], F32, name="ident32")
    dma(ident32[:], ident.ap())
    identr = cst.tile([128, 128], F32R, name="identr")
    dma(identr[:], identrd.ap())
    ones_r32 = cst.tile([1, D], F32, name="ones_r32")
    dma(ones_r32[:], onesr.ap())
    ones_rr = cst.tile([1, D], F32R, name="ones_rr")
    dma(ones_rr[:], onesrd.ap())
    ones_cr = cst.tile([128, 1], F32R, name="ones_cr")
    dma(ones_cr[:], onescd.ap())
    iota_row_sb = cst.tile([UP, L], F32, name="iota_row_sb")
    dma(iota_row_sb[:], iota_row.ap())
    iotac_sb = cst.tile([128, H * UP], F32, name="iotac_sb")
    dma(iotac_sb[:], iotac.ap())

    # ---- mid-life tensor pools (creation order = reverse release order) ----
    vr_cm, vr_p = pool("vrp")            # released after phase 5b
    v_r = [vr_p.tile([128, D], F32R, name=f"vr{i}") for i in range(NL)]
    vmean_r = vr_p.tile([1, D], F32R, name="vmean_r")
    negvm_r = vr_p.tile([1, D], F32R, name="negvm_r")
    kThi_cm, kThi_p = pool("kThi")       # released after phase 5
    kT32 = [kThi_p.tile([128, L], F32, name=f"kT32_{i}") for i in range(ND)]
    q32_cm, q32_p = pool("q32p")         # released after phase 5
    q32 = [q32_p.tile([128, D], F32, name=f"q32_{i}") for i in range(NL)]
    mtM_cm, mtM_p = pool("mtM")          # released in phase 4
    mt_sb = [mtM_p.tile([128, H], F32, name=f"mt{i}") for i in range(NL)]
    M_sb = [mtM_p.tile([128, H], F32, name=f"M{i}") for i in range(NL)]
    kr_cm, kr_p = pool("krp")            # released after phase 3
    k32 = [kr_p.tile([128, D], F32, name=f"k32_{i}") for i in range(NL)]
    qkx_cm, qkx = pool("qkx")            # released after phase 2
    qT32 = [qkx.tile([128, L], F32, name=f"qT32_{i}") for i in range(ND)]

    # =================== phase 1: splits + projections ===================
    with tc.tile_pool(name="p1", bufs=1) as p1, \
         tc.tile_pool(name="tmp1", bufs=1) as tmp1, \
         tc.tile_pool(name="ps1", bufs=4, space="PSUM") as ps1, \
         tc.tile_pool(name="psA", bufs=1, space="PSUM") as psA:
        x_hi = [p1.tile([128, L], F32R, name=f"xh{i}") for i in range(ND)]
        x_lo = [p1.tile([128, L], F32R, name=f"xl{i}") for i in range(ND)]
        for i in range(ND):
            t32 = tmp1.tile([128, L], F32, tag="t32x", name=f"xT32_{i}")
            dma(t32[:], xT.ap()[i * 128:(i + 1) * 128, :])
            nc.scalar.copy(x_hi[i][:], t32[:])
            nc.vector.tensor_tensor(
                out=x_lo[i][:], in0=t32[:], in1=x_hi[i][:], op=OP.subtract)
            if dbg:
                dma(DBG_XH.ap()[i * 128:(i + 1) * 128, :], x_hi[i][:])
                dma(DBG_XL.ap()[i * 128:(i + 1) * 128, :], x_lo[i][:])

        for nm, Wd, tsink, sink, cvt in (
                ("q", Wq, qT32, q32, "dve32"),
                ("k", Wk, kT32, k32, "actr")):
            with tc.tile_pool(name=f"w{nm}", bufs=1) as wp:
                w_hi, w_lo = [], []
                for i in range(ND):
                    w32 = tmp1.tile([128, D], F32, tag="t32w",
                                    name=f"w32_{nm}{i}")
                    dma(w32[:], Wd.ap()[i * 128:(i + 1) * 128, :])
                    hi = wp.tile([128, D], F32R, name=f"whi_{nm}{i}")
                    lo = wp.tile([128, D], F32R, name=f"wlo_{nm}{i}")
                    nc.scalar.copy(hi[:], w32[:])
                    nc.vector.tensor_tensor(out=lo[:], in0=w32[:], in1=hi[:],
                                            op=OP.subtract)
                    w_hi.append(hi)
                    w_lo.append(lo)
                    if dbg and nm == "q":
                        dma(DBG_WH.ap()[i * 128:(i + 1) * 128, :], hi[:])
                        dma(DBG_WL.ap()[i * 128:(i + 1) * 128, :], lo[:])
                # transposed layout [hd, l]: 3-pass exact -> hi/lo splits
                for dk in range(ND):
                    for l2 in range(2):
                        ps = ps1.tile([128, 512], F32, tag="proj",
                                      name="proj_ps")
                        nmm = 0
                        for dd in range(ND):
                            for wset, xset in ((w_hi, x_hi), (w_hi, x_lo),
                                               (w_lo, x_hi)):
                                nc.tensor.matmul(
                                    ps[:],
                                    wset[dd][:, dk * 128:(dk + 1) * 128],
                                    xset[dd][:, l2 * 512:(l2 + 1) * 512],
                                    start=(nmm == 0), stop=(nmm == 11))
                                nmm += 1
                        osl = tsink[dk][:, l2 * 512:(l2 + 1) * 512]
                        if nm == "q":
                            nc.vector.tensor_copy(osl, ps[:])
                        else:
                            nc.scalar.copy(osl, ps[:])
                # normal layout [l, hd]: 3-pass exact
                for lt in range(NL):
                    ps = ps1.tile([128, 512], F32, tag="proj", name="qkn_ps")
                    nmm = 0
                    for dd in range(ND):
                        for wset, xset in ((w_hi, x_hi), (w_hi, x_lo),
                                           (w_lo, x_hi)):
                            nc.tensor.matmul(
                                ps[:],
                                xset[dd][:, lt * 128:(lt + 1) * 128],
                                wset[dd][:],
                                start=(nmm == 0), stop=(nmm == 11))
                            nmm += 1
                    if cvt == "dve32":
                        nc.vector.tensor_copy(sink[lt][:], ps[:])
                    else:
                        nc.scalar.copy(sink[lt][:], ps[:])
        if dbg:
            for dk in range(ND):
                dma(DBG_QT.ap()[dk * 128:(dk + 1) * 128, :], qT32[dk][:])

        # v: 1-pass fp32r, normal layout [l, hd]
        with tc.tile_pool(name="wv", bufs=1) as wvp:
            Wv_r = []
            for i in range(ND):
                wv = wvp.tile([128, D], F32R, name=f"wv{i}")
                dma(wv[:], Wv.ap()[i * 128:(i + 1) * 128, :])
                Wv_r.append(wv)
            for lt in range(NL):
                ps = ps1.tile([128, 512], F32, tag="proj", name="v_ps")
                for dd in range(ND):
                    nc.tensor.matmul(
                        ps[:],
                        x_hi[dd][:, lt * 128:(lt + 1) * 128],
                        Wv_r[dd][:],
                        start=(dd == 0), stop=(dd == ND - 1))
                nc.scalar.copy(v_r[lt][:], ps[:])

        # vmean row [1, 512] and its negation (fp32r rows for K=1 matmuls)
        psv = psA.tile([1, 512], F32, tag="vm", name="vm_ps")
        for lt in range(NL):
            nc.tensor.matmul(psv[:], ones_cr[:], v_r[lt][:],
                             start=(lt == 0), stop=(lt == NL - 1))
        nc.scalar.mul(vmean_r[:], psv[:], 1.0 / L)
        nc.scalar.mul(negvm_r[:], psv[:], -1.0 / L)

    if stages < 2:
        for cm in (qkx_cm, kr_cm, mtM_cm, q32_cm, kThi_cm, vr_cm, cst_cm):
            cm.__exit__(None, None, None)
        return

    # =================== phase 2: QK + masked max ===================
    # The sampling mask (0 / -1e30) is injected into PSUM via an identity
    # matmul as the first accumulation step; a single reduce_max per
    # (head, l-tile) then yields the masked max.
    with tc.tile_pool(name="mask", bufs=2) as maskp, \
         tc.tile_pool(name="psQK", bufs=3, space="PSUM") as psQK:
        for lt in range(NL):
            mtile = maskp.tile([128, L], F32R, tag="mask", name=f"mask{lt}")
            dma(mtile[:], maskadd.ap()[lt * 128:(lt + 1) * 128, :])
            lsl = slice(lt * 128, (lt + 1) * 128)
            for hp in range(4):  # heads (2hp, 2hp+1) live in qT tile hp
                for hh in range(2):
                    h = 2 * hp + hh
                    rsl = slice(hh * 64, hh * 64 + 64)
                    tp = (hh * 64, 0)
                    psh = psQK.tile([128, L], F32, tag="qk", name=f"qk{h}")
                    for j2 in range(2):
                        jsl = slice(j2 * 512, (j2 + 1) * 512)
                        nc.tensor.matmul(
                            psh[:, jsl], identr[:], mtile[:, jsl],
                            start=True, stop=False)
                        nc.tensor.matmul(
                            psh[:, jsl],
                            qT32[hp][rsl, lsl],
                            kT32[hp][rsl, jsl],
                            start=False, stop=True,
                            tile_position=tp)
                    nc.vector.reduce_max(
                        mt_sb[lt][:, h:h + 1], psh[:], axis=AX.X)
    qkx_cm.__exit__(None, None, None)

    # =================== phase 3: kbar + mean -> M ===================
    with tc.tile_pool(name="cnt", bufs=2) as cntp, \
         tc.tile_pool(name="kbp", bufs=1) as kbp, \
         tc.tile_pool(name="escr", bufs=2) as escrp, \
         tc.tile_pool(name="psKB", bufs=1, space="PSUM") as psKB:
        kbar32 = [kbp.tile([128, D], F32, name=f"kb{i}") for i in range(NL)]
        kb_ps = [psKB.tile([128, 512], F32, tag=f"kbps{i}", name=f"kbps{i}")
                 for i in range(NL)]
        for jt in range(NJ):
            ct = cntp.tile([128, L], F32, tag="cnt", name=f"cnt{jt}")
            dma(ct[:], CntT.ap()[jt * 128:(jt + 1) * 128, :])
            for lt in range(NL):
                nc.tensor.matmul(
                    kb_ps[lt][:],
                    ct[:, lt * 128:(lt + 1) * 128],
                    k32[jt][:],
                    start=(jt == 0), stop=(jt == NJ - 1))
        for lt in range(NL):
            nc.vector.tensor_copy(kbar32[lt][:], kb_ps[lt][:])
        for lt in range(NL):
            E = escrp.tile([128, D], F32, tag="E", name="E")
            nc.vector.tensor_tensor(out=E[:], in0=q32[lt][:],
                                    in1=kbar32[lt][:], op=OP.mult)
            Ered = escrp.tile([128, H], F32, tag="Ered", name="Ered")
            nc.vector.tensor_reduce(
                out=Ered[:], in_=E[:].rearrange("p (h d) -> p h d", d=DK),
                axis=AX.X, op=OP.add)
            nc.vector.scalar_tensor_tensor(
                out=M_sb[lt][:], in0=Ered[:], scalar=-1.0 / U,
                in1=mt_sb[lt][:], op0=OP.mult, op1=OP.add)
            if dbg:
                dma(DBG_M.ap()[lt * 128:(lt + 1) * 128, :], M_sb[lt][:])
                dma(DBG_MT2.ap()[lt * 128:(lt + 1) * 128, :], mt_sb[lt][:])
    kr_cm.__exit__(None, None, None)

    # =================== phase 4: topk(M) -> m_top ===================
    psT_cm = tc.tile_pool(name="psT", bufs=1, space="PSUM")
    psT = psT_cm.__enter__()
    if True:
        psMT = psT.tile([H, L], F32, tag="mtT", name="mtT_ps")
        for lt in range(NL):
            nc.tensor.matmul(psMT[:, lt * 128:(lt + 1) * 128],
                             M_sb[lt][:], ident32[:],
                             is_transpose=True, start=True, stop=True)
        mtM_cm.__exit__(None, None, None)
        oaT_cm, oaT_p = pool("oaTp")     # reserved early for LIFO nesting
        oaT = [oaT_p.tile([128, L], F32R, name=f"oa{i}") for i in range(ND)]
        atn_cm, atn = pool("atn")
        MTa = atn.tile([H, L], F32, name="MTa")
        MTb = atn.tile([H, L], F32, name="MTb")
        nc.vector.tensor_copy(MTa[:], psMT[:])
        topv = atn.tile([H, 40], F32, name="topv")
        topi = atn.tile([H, 40], U32, name="topi")
        cur, nxt = MTa, MTb
        for r in range(5):
            vs = topv[:, r * 8:(r + 1) * 8]
            nc.vector.max(vs, cur[:])
            nc.vector.max_index(topi[:, r * 8:(r + 1) * 8], vs, cur[:])
            if r < 4:
                nc.vector.match_replace(nxt[:], vs, cur[:], NEG)
                cur, nxt = nxt, cur
        mtop_i = atn.tile([H, U], I32, name="mtop_i")
        nc.vector.tensor_copy(mtop_i[:], topi[:, 0:U])
        dma(MTOP.ap(), mtop_i[:])
        mtop_f = atn.tile([H, UP], F32, name="mtop_f")
        nc.vector.memset(mtop_f[:], -1.0)
        nc.vector.tensor_copy(mtop_f[:, 0:U], topi[:, 0:U])
        # bounce via DRAM for the [36, 8] transpose and [1, 288] flat row
        dma(mscr.ap().rearrange("x (a b) -> (x a) b", a=H, b=UP), mtop_f[:])
        m_topT = atn.tile([UP, H], F32, name="m_topT")
        dma(m_topT[:], mscr.ap().rearrange("x (a b) -> (x b) a", a=H, b=UP))
        mbc_row = atn.tile([1, H * UP], F32, name="mbc_row")
        dma(mbc_row[:], mscr.ap())
        psbc = psT.tile([128, H * UP], F32, tag="mbc", name="mbc_ps")
        nc.tensor.matmul(psbc[:], ones_r32[:, 0:128], mbc_row[:],
                         start=True, stop=True)
        mbc = atn.tile([128, H * UP], F32, name="mbc")
        nc.vector.tensor_copy(mbc[:], psbc[:])
    psT_cm.__exit__(None, None, None)

    # ============ phase 5: S matrices, q_red, scores, softmax ============
    with tc.tile_pool(name="psS", bufs=2, space="PSUM") as psS, \
         tc.tile_pool(name="psSC", bufs=2, space="PSUM") as psSC:
        S_T = [atn.tile([128, H * UP], F32, name=f"ST{j}") for j in range(NJ)]
        for jt in range(NJ):
            nc.vector.scalar_tensor_tensor(
                out=S_T[jt][:], in0=mbc[:], scalar=float(-jt * 128),
                in1=iotac_sb[:], op0=OP.add, op1=OP.is_equal)
        qred_r = [atn.tile([128, UP], F32, name=f"qredp{hp}")
                  for hp in range(4)]
        for h in range(H):
            psq = psS.tile([64, UP], F32, tag="qred", name="qred_ps")
            for lt in range(NL):
                nc.tensor.matmul(
                    psq[:],
                    q32[lt][:, h * DK:(h + 1) * DK],
                    S_T[lt][:, h * UP:(h + 1) * UP],
                    start=(lt == 0), stop=(lt == NL - 1))
            nc.scalar.copy(
                qred_r[h // 2][(h % 2) * 64:(h % 2) * 64 + 64, :], psq[:])
        rsum = [atn.tile([UP, 1], F32, name=f"rs{h}") for h in range(H)]
        recip = [atn.tile([UP, 1], F32, name=f"rc{h}") for h in range(H)]
        attn_r = [atn.tile([UP, L], F32R, name=f"at{h}") for h in range(H)]
        for h in range(H):
            pssc = psSC.tile([UP, L], F32, tag="scores", name="sc_ps")
            for l2 in range(2):
                nc.tensor.matmul(
                    pssc[:, l2 * 512:(l2 + 1) * 512],
                    qred_r[h // 2][(h % 2) * 64:(h % 2) * 64 + 64, :],
                    kT32[h // 2][(h % 2) * 64:(h % 2) * 64 + 64,
                                 l2 * 512:(l2 + 1) * 512],
                    start=True, stop=True)
            rmax = atn.tile([UP, 1], F32, tag="rmax", name="rmax", bufs=2)
            nc.vector.reduce_max(rmax[:], pssc[:], axis=AX.X)
            nmax = atn.tile([UP, 1], F32, tag="nmax", name="nmax", bufs=2)
            nc.vector.tensor_scalar_mul(nmax[:], rmax[:], -0.125)
            nc.scalar.activation(
                attn_r[h][:], pssc[:], ACTF.Exp,
                bias=nmax[:], scale=0.125, accum_out=rsum[h][:])
            nc.vector.reciprocal(recip[h][:], rsum[h][:])
            nc.vector.tensor_scalar(
                out=attn_r[h][:], in0=attn_r[h][:], scalar1=recip[h][:],
                scalar2=None, op0=OP.mult)
    # ============ phase 5b: attn^T, upd/delta, ctx^T ============
    with tc.tile_pool(name="psAT", bufs=2, space="PSUM") as psAT:
        attnT = [atn.tile([128, H * UP], F32R, name=f"aT{lt}")
                 for lt in range(NL)]
        for lt in range(NL):
            psa = psAT.tile([128, H * UP], F32, tag="aT", name="aT_ps")
            for h in range(H):
                nc.tensor.matmul(
                    psa[:, h * UP:(h + 1) * UP],
                    attn_r[h][:, lt * 128:(lt + 1) * 128],
                    identr[0:UP, 0:UP],
                    start=True, stop=True)
            nc.scalar.copy(attnT[lt][:], psa[:])
        delta_r = atn.tile([UP, H * DV], F32R, name="delta_r")
        for h in range(H):
            psu = psAT.tile([UP, DV], F32, tag="upd", name="upd_ps")
            for lt in range(NL):
                nc.tensor.matmul(
                    psu[:],
                    attnT[lt][:, h * UP:(h + 1) * UP],
                    v_r[lt][:, h * DV:(h + 1) * DV],
                    start=(lt == 0), stop=False)
            nc.tensor.matmul(psu[:], ones_rr[:, 0:UP],
                             negvm_r[:, h * DV:(h + 1) * DV],
                             start=False, stop=True)
            nc.scalar.copy(delta_r[:, h * DV:(h + 1) * DV], psu[:])
        ctxT = [atn.tile([128, L], F32R, name=f"cx{i}") for i in range(ND)]
        srp_cm = tc.tile_pool(name="srp", bufs=2)
        srp = srp_cm.__enter__()
        for h in range(H):
            S_h = srp.tile([UP, L], F32R, tag="Sh", name=f"S{h}")
            nc.vector.tensor_scalar(
                out=S_h[:], in0=iota_row_sb[:],
                scalar1=m_topT[:, h:h + 1], scalar2=None, op0=OP.is_equal)
            for l2 in range(2):
                psc = psAT.tile([64, 512], F32, tag="ctx", name="ctx_ps")
                nc.tensor.matmul(
                    psc[:],
                    delta_r[:, h * DV:(h + 1) * DV],
                    S_h[:, l2 * 512:(l2 + 1) * 512],
                    start=True, stop=False)
                nc.tensor.matmul(
                    psc[:],
                    vmean_r[:, h * DV:(h + 1) * DV],
                    ones_rr[:, 0:512],
                    start=False, stop=True)
                nc.scalar.copy(
                    ctxT[h // 2][(h % 2) * 64:(h % 2) * 64 + 64,
                                 l2 * 512:(l2 + 1) * 512],
                    psc[:])
        srp_cm.__exit__(None, None, None)

    # =================== phase 6: Wo -> attn_out^T ===================
    with tc.tile_pool(name="wo", bufs=1) as wop, \
         tc.tile_pool(name="psWO", bufs=4, space="PSUM") as psWO:
        Wo_r = []
        for i in range(ND):
            w = wop.tile([128, D], F32R, name=f"wo{i}")
            dma(w[:], Wo.ap()[i * 128:(i + 1) * 128, :])
            Wo_r.append(w)
        for dt_ in range(ND):
            for l2 in range(2):
                pso = psWO.tile([128, 512], F32, tag="oa", name="oa_ps")
                for hc in range(ND):
                    nc.tensor.matmul(
                        pso[:],
                        Wo_r[hc][:, dt_ * 128:(dt_ + 1) * 128],
                        ctxT[hc][:, l2 * 512:(l2 + 1) * 512],
                        start=(hc == 0), stop=(hc == ND - 1))
                nc.scalar.copy(oaT[dt_][:, l2 * 512:(l2 + 1) * 512], pso[:])
                if dbg:
                    dmac(DBG_AO.ap()[dt_ * 128:(dt_ + 1) * 128,
                                     l2 * 512:(l2 + 1) * 512],
                         oaT[dt_][:, l2 * 512:(l2 + 1) * 512])
    atn_cm.__exit__(None, None, None)

    # ============ phase 7: FFN + residual + layernorm ============
    with tc.tile_pool(name="ffn", bufs=1) as ffn, \
         tc.tile_pool(name="hTp", bufs=1) as hTp, \
         tc.tile_pool(name="lnp", bufs=2) as lnp, \
         tc.tile_pool(name="psF", bufs=3, space="PSUM") as psF:
        c1_r = []
        for i in range(ND):
            w = ffn.tile([128, DFF], F32R, name=f"c1_{i}")
            dma(w[:], c1T.ap()[i * 128:(i + 1) * 128, :])
            c1_r.append(w)
        c2_r = []
        for i in range(NF):
            w = ffn.tile([128, D], F32R, name=f"c2_{i}")
            dma(w[:], c2T.ap()[i * 128:(i + 1) * 128, :])
            c2_r.append(w)
        for l2 in range(2):
            hT = [hTp.tile([128, 512], F32R, tag=f"hT{i}", name=f"hT{l2}_{i}")
                  for i in range(NF)]
            for ft in range(NF):
                psh = psF.tile([128, 512], F32, tag="h", name="h_ps")
                for dd in range(ND):
                    nc.tensor.matmul(
                        psh[:],
                        c1_r[dd][:, ft * 128:(ft + 1) * 128],
                        oaT[dd][:, l2 * 512:(l2 + 1) * 512],
                        start=(dd == 0), stop=(dd == ND - 1))
                nc.scalar.activation(hT[ft][:], psh[:], ACTF.Gelu)
            for li in range(4):
                lt = l2 * 4 + li
                psz = psF.tile([128, 512], F32, tag="z", name="z_ps")
                for ft in range(NF):
                    nc.tensor.matmul(
                        psz[:],
                        hT[ft][:, li * 128:(li + 1) * 128],
                        c2_r[ft][:],
                        start=(ft == 0), stop=False)
                for dd in range(ND):
                    nc.tensor.matmul(
                        psz[:, dd * 128:(dd + 1) * 128],
                        oaT[dd][:, lt * 128:(lt + 1) * 128],
                        identr[:],
                        start=False, stop=(dd == ND - 1))
                stats = lnp.tile([128, 6], F32, tag="st", name="st")
                aggr = lnp.tile([128, 2], F32, tag="ag", name="ag")
                nc.vector.bn_stats(stats[:], psz[:])
                nc.vector.bn_aggr(aggr[:], stats[:])
                veps = lnp.tile([128, 1], F32, tag="veps", name="veps")
                nc.vector.tensor_scalar_add(veps[:], aggr[:, 1:2], 1e-5)
                std = lnp.tile([128, 1], F32, tag="std", name="std")
                nc.scalar.sqrt(std[:], veps[:])
                rstd = lnp.tile([128, 1], F32, tag="rstd", name="rstd")
                nc.vector.reciprocal(rstd[:], std[:])
                o = lnp.tile([128, D], F32, tag="o", name="o")
                nc.vector.tensor_scalar(
                    out=o[:], in0=psz[:], scalar1=aggr[:, 0:1], scalar2=rstd[:],
                    op0=OP.subtract, op1=OP.mult)
                dma(OUT.ap()[lt * 128:(lt + 1) * 128, :], o[:])
    oaT_cm.__exit__(None, None, None)
    q32_cm.__exit__(None, None, None)
    kThi_cm.__exit__(None, None, None)
    vr_cm.__exit__(None, None, None)
    cst_cm.__exit__(None, None, None)


_BUILD_CACHE = {}


def _build(dbg=False):
    key = ("nc", dbg)
    if key in _BUILD_CACHE:
        return _BUILD_CACHE[key]
    nc = bass.Bass("TRN2", target_bir_lowering=False, debug=False)
    with tile.TileContext(nc, pool_alloc_mode="queue") as tc:
        _emit(nc, tc, dbg=dbg)
    _BUILD_CACHE[key] = nc
    return nc


def _host_prep(inputs):
    """Per-core input maps. Numpy only: layout/index transforms, no math."""
    x = np.ascontiguousarray(np.asarray(inputs["x"], dtype=np.float32))
    idx = np.asarray(inputs["index_sample"]).astype(np.int64)

    for nm in ("bq", "bk", "bv", "bo", "conv1_b", "conv2_b"):
        if nm in inputs and not np.allclose(np.asarray(inputs[nm]), 0.0):
            raise NotImplementedError(f"nonzero bias {nm} not supported")

    cnt = np.zeros((L, L), dtype=np.float32)
    np.add.at(cnt, (np.repeat(np.arange(L), idx.shape[1]), idx.reshape(-1)),
              1.0)
    CntT = np.ascontiguousarray(cnt.T)
    maskadd = np.where(cnt > 0, np.float32(0.0), np.float32(MASKNEG))

    shared = {
        "Wq": np.asarray(inputs["Wq"], dtype=np.float32),
        "Wk": np.asarray(inputs["Wk"], dtype=np.float32),
        "Wv": np.asarray(inputs["Wv"], dtype=np.float32),
        "Wo": np.asarray(inputs["Wo"], dtype=np.float32),
        "c1T": np.ascontiguousarray(
            np.asarray(inputs["conv1_w"], dtype=np.float32).T),
        "c2T": np.ascontiguousarray(
            np.asarray(inputs["conv2_w"], dtype=np.float32).T),
        "CntT": CntT,
        "maskadd": maskadd,
        "iota_row": np.broadcast_to(
            np.arange(L, dtype=np.float32), (UP, L)).copy(),
        "iotac": np.broadcast_to(
            np.arange(128, dtype=np.float32)[:, None], (128, H * UP)).copy(),
        "ident": np.eye(128, dtype=np.float32),
        "identrd": np.eye(128, dtype=np.float32),
        "onesr": np.ones((1, D), dtype=np.float32),
        "onesrd": np.ones((1, D), dtype=np.float32),
        "onesc": np.ones((128, 1), dtype=np.float32),
        "onescd": np.ones((128, 1), dtype=np.float32),
    }
    in_maps = []
    for b in range(B):
        m = dict(shared)
        m["xT"] = np.ascontiguousarray(x[b].T)
        in_maps.append(m)
    return in_maps


def _postprocess(out, gamma, beta):
    g = np.asarray(gamma, dtype=np.float32)
    bta = np.asarray(beta, dtype=np.float32)
    if not np.allclose(g, 1.0) or not np.allclose(bta, 0.0):
        out = out * g + bta
    return out


def kernel(**inputs):
    nc = split_multi_waits(_build(dbg=False))
    in_maps = _host_prep(inputs)
    res = bass_utils.run_bass_kernel_spmd(nc, in_maps, list(range(B)))
    out = np.stack([res.results[b]["OUT"] for b in range(B)])
    out = _postprocess(out, inputs["gamma"], inputs["beta"])
    m_top = np.stack([res.results[b]["MTOP"] for b in range(B)]).astype(np.int32)
    return out, m_top, m_top
